# revision 26
# baseline (speedup 1.0000x reference)
"""Trainium2 Bass kernel for nn_Attention_35588099015470.

Full attention block: LoRA linears (folded host-side) + RoPE + causal SDPA +
output projection. B=2 T=2048 C=2048 H=16 D=128, fp32 in/out.

Sharding: hybrid 2 (batch) x 4 (head-group). Core c handles batch c//4 and
heads 4*(c%4)..4*(c%4)+3, so each core loads only its batch's activations.

All matmul operands are bf16 (host-cast; fp32 PSUM accumulation) — same PE
rate as fp32r but half the HBM traffic and SBUF footprint, which lets q/k/v
stay SBUF-resident between projection and attention. v is produced directly
in natural [token, feat] layout by using the x tile as the matmul stationary,
so no PE transposes are needed.

Emission interleaves the three stages per 512-token tile —
A(t) projection+RoPE, att(t) causal attention, then the output-projection
round C(t-1) — so the per-round AllToAll (which re-shards head-parallel y to
token-parallel) completes in the shadow of the next tile's projection GEMMs
and the PE never waits on the collective. Attention keeps scores in
[key, query] layout: softmax denominators via an all-ones stationary matmul,
causal masking as a 0/1 vector-engine multiply, normalization as a deferred
reciprocal(approx)+broadcast+multiply. The C rounds keep the moving operand
512 wide (stationary = incoming activations, moving = projection weight) so
LDWEIGHTS stays pipelined behind the matmul stream.

Biases are guaranteed zero by the problem's setup_inputs and the mask is the
causal tril; if either assumption is violated at runtime we fall back to a
host reference implementation so the kernel stays correct on any input.
"""
import sys

sys.path.insert(0, "/opt/trn_rl_repo")

import numpy as np
import ml_dtypes
from contextlib import ExitStack

import concourse.tile as tile
from concourse import bacc, mybir
from concourse.bass_utils import run_bass_kernel_spmd

dt = mybir.dt
BF = dt.bfloat16

B, T, C, H, R = 2, 2048, 2048, 16, 8
D = C // H            # 128
NCORES = 8
HPC = 4               # heads per core
P = 128
KC = C // P           # 16 contraction chunks
QT = T // 512         # 4 query rounds / token tiles
SCALE = 1.0 / float(np.sqrt(D))

_PROGRAM = None


def _build_program():
    nc = bacc.Bacc("TRN2", target_bir_lowering=False, debug=False,
                   num_devices=NCORES)

    xT_d = nc.dram_tensor("xT", [C, T], BF, kind="ExternalInput")
    wqT_d = nc.dram_tensor("wqT", [C, HPC * D], BF, kind="ExternalInput")
    wkT_d = nc.dram_tensor("wkT", [C, HPC * D], BF, kind="ExternalInput")
    wvT_d = nc.dram_tensor("wvT", [C, HPC * D], BF, kind="ExternalInput")
    pwM_d = nc.dram_tensor("pwM", [P, KC, C], BF, kind="ExternalInput")
    cosA_d = nc.dram_tensor("cosA", [P, T], dt.float32, kind="ExternalInput")
    sinA_d = nc.dram_tensor("sinA", [P, T], dt.float32, kind="ExternalInput")
    binm_d = nc.dram_tensor("binm", [P, 896], BF, kind="ExternalInput")

    outN_d = nc.dram_tensor("outN", [512, C], BF, kind="ExternalOutput")

    with tile.TileContext(nc) as tc, ExitStack() as ctx:
        dram = ctx.enter_context(tc.tile_pool(name="dram", bufs=1, space="DRAM"))
        # rounds 0-2: one A2A each; round 3 split in two (tail latency)
        chs = [dram.tile([NCORES, HPC * D, 64], BF, name=f"ch_{r}")
               for r in range(QT - 1)]
        yos = [dram.tile([NCORES, HPC * D, 64], BF, name=f"yo_{r}")
               for r in range(QT - 1)]
        ch3 = [dram.tile([NCORES, 2 * D, 64], BF, name=f"ch3_{i}")
               for i in range(2)]
        yo3 = [dram.tile([NCORES, 2 * D, 64], BF, name=f"yo3_{i}")
               for i in range(2)]

        cst = ctx.enter_context(tc.tile_pool(name="cst", bufs=1))
        kvp = ctx.enter_context(tc.tile_pool(name="kvp", bufs=1))
        wp = ctx.enter_context(tc.tile_pool(name="wp", bufs=1))
        xp = ctx.enter_context(tc.tile_pool(name="xp", bufs=2))
        csp = ctx.enter_context(tc.tile_pool(name="csp", bufs=2))
        qp = ctx.enter_context(tc.tile_pool(name="qp", bufs=1))
        tp = ctx.enter_context(tc.tile_pool(name="tp", bufs=2))
        ptp = ctx.enter_context(tc.tile_pool(name="ptp", bufs=1))
        yp = ctx.enter_context(tc.tile_pool(name="yp", bufs=2))
        ycp = ctx.enter_context(tc.tile_pool(name="ycp", bufs=1))
        ocp = ctx.enter_context(tc.tile_pool(name="ocp", bufs=2))
        pp = ctx.enter_context(tc.tile_pool(name="pp", bufs=1, space="PSUM"))

        ones_f = cst.tile([P, P], dt.float32, name="ones_f")
        nc.any.memset(ones_f[:], 1.0)
        ones_r = cst.tile([P, P], BF, name="ones_r")
        nc.vector.tensor_copy(ones_r[:], ones_f[:])
        binm = cst.tile([P, 896], BF, name="binm")

        k_t = [kvp.tile([P, HPC, 512], BF, name=f"k_{t}") for t in range(QT)]
        v_t = [kvp.tile([P, 4, HPC * D], BF, name=f"v_{t}") for t in range(QT)]

        xT_view = xT_d.ap().rearrange("(a p) t -> p a t", p=P)
        wq_sb = wp.tile([P, KC, HPC * D], BF, name="wq_sb")
        wk_sb = wp.tile([P, KC, HPC * D], BF, name="wk_sb")
        wv_sb = wp.tile([P, KC, HPC * D], BF, name="wv_sb")
        pw_sb = wp.tile([P, KC, C], BF, name="pw_sb")

        # DMA order: wq + first x tile first so the PE starts ASAP
        wqv = wqT_d.ap().rearrange("(a p) m -> p a m", p=P)
        xt0 = xp.tile([P, KC, 512], BF, tag="xt", name="xt_0")
        for g in range(4):
            nc.sync.dma_start(wq_sb[:, g * 4:(g + 1) * 4, :],
                              wqv[:, g * 4:(g + 1) * 4, :])
            nc.sync.dma_start(xt0[:, g * 4:(g + 1) * 4, :],
                              xT_view[:, g * 4:(g + 1) * 4, 0:512])
        cc0 = csp.tile([P, 512], dt.float32, tag="cc", name="cc_0")
        nc.sync.dma_start(cc0[:], cosA_d.ap()[:, 0:512])
        ss0 = csp.tile([P, 512], dt.float32, tag="ss", name="ss_0")
        nc.sync.dma_start(ss0[:], sinA_d.ap()[:, 0:512])
        for w_sb, wd in ((wk_sb, wkT_d), (wv_sb, wvT_d)):
            wvw = wd.ap().rearrange("(a p) m -> p a m", p=P)
            for g in range(4):
                nc.sync.dma_start(w_sb[:, g * 4:(g + 1) * 4, :],
                                  wvw[:, g * 4:(g + 1) * 4, :])
        nc.sync.dma_start(binm[:], binm_d.ap())
        for g in range(4):
            nc.sync.dma_start(pw_sb[:, g * 4:(g + 1) * 4, :],
                              pwM_d.ap()[:, g * 4:(g + 1) * 4, :])

        def load_half(r, half):
            yh = ycp.tile([P, 8, 128], BF, tag="yh", bufs=2,
                          name=f"yh_{r}_{half}")
            if r < QT - 1:
                for i in range(NCORES):
                    hgi, bh = i % 4, i // 4
                    sap = yos[r][i].rearrange("(a p) t -> p a t", p=P)[
                        :, 2 * half:2 * half + 2, :]
                    nc.sync.dma_start(
                        yh[:, 2 * hgi:2 * hgi + 2, bh * 64:(bh + 1) * 64],
                        sap)
            else:
                for bh in range(2):
                    nc.sync.dma_start(
                        yh[:, :, bh * 64:(bh + 1) * 64],
                        yo3[half][4 * bh:4 * bh + 4].rearrange(
                            "s (a p) t -> p (s a) t", p=P))
            return yh

        def emit_C(r, pre=None):
            # two half tiles per round (heads 0-1 / 2-3 of each src) so the
            # last round's first half is not gated on its second A2A
            yhs = list(pre) if pre else []
            for half in range(len(yhs), 2):
                yhs.append(load_half(r, half))

            def half_mms(cps, cb, half):
                for j in range(8):
                    nc.tensor.matmul(
                        cps[:], yhs[half][:, j, :],
                        pw_sb[:, 4 * (j // 2) + 2 * half + (j % 2),
                              cb * 512:(cb + 1) * 512],
                        start=(half == 0 and j == 0),
                        stop=(half == 1 and j == 7))

            def drain(cb, cps):
                osb = ocp.tile([P, 512], BF, tag="osb", bufs=2,
                               name=f"osb_{r}_{cb}")
                nc.scalar.copy(osb[:], cps[:])
                nc.sync.dma_start(
                    outN_d.ap()[r * P:(r + 1) * P,
                                cb * 512:(cb + 1) * 512],
                    osb[:])

            if r == QT - 1:
                # last round: all first halves run before the final A2A
                # lands, borrowing the now-idle pv psum buffers for cb2/3
                cts = {}
                for cb in range(4):
                    cts[cb] = pp.tile([P, 512], dt.float32,
                                      tag=("a" if cb < 2 else "pv"), bufs=2,
                                      name=f"cps_{r}_{cb}")
                    half_mms(cts[cb], cb, 0)
                for cb in range(4):
                    half_mms(cts[cb], cb, 1)
                for cb in range(4):
                    drain(cb, cts[cb])
            else:
                for pair in range(2):
                    cbs = (2 * pair, 2 * pair + 1)
                    cts = {}
                    for cb in cbs:
                        cts[cb] = pp.tile([P, 512], dt.float32, tag="a",
                                          bufs=2, name=f"cps_{r}_{cb}")
                        half_mms(cts[cb], cb, 0)
                    for cb in cbs:
                        half_mms(cts[cb], cb, 1)
                    for cb in cbs:
                        drain(cb, cts[cb])

        nxt = {0: (xt0, cc0, ss0)}
        for tt in range(QT):
            tsl = slice(tt * 512, (tt + 1) * 512)
            # ---- A(tt): q/k/v projections + rope for this token tile ----
            xt, cc, ss = nxt.pop(tt)
            qtile = qp.tile([P, HPC, 512], BF, tag="qt", name=f"q_{tt}")
            for h in range(HPC):
                for w_sb, dst in ((wq_sb, qtile), (wk_sb, k_t[tt])):
                    ps = pp.tile([P, 512], dt.float32, tag="a", bufs=2,
                                 name=f"psA_{tt}_{h}")
                    for kc in range(KC):
                        nc.tensor.matmul(
                            ps[:], w_sb[:, kc, h * P:(h + 1) * P],
                            xt[:, kc, :],
                            start=(kc == 0), stop=(kc == KC - 1))
                    # rope: y = raw*cosA + halfswap(raw)*sinA
                    t1 = tp.tile([P, 512], dt.float32, tag="tr", bufs=3,
                                 name=f"t1_{tt}_{h}")
                    nc.vector.tensor_mul(t1[:], ps[:], cc[:])
                    t2 = tp.tile([P, 512], dt.float32, tag="tr", bufs=3,
                                 name=f"t2_{tt}_{h}")
                    nc.vector.tensor_mul(t2[0:64, :], ps[64:128, :],
                                         ss[0:64, :])
                    nc.vector.tensor_mul(t2[64:128, :], ps[0:64, :],
                                         ss[64:128, :])
                    nc.vector.tensor_add(dst[:, h, :], t1[:], t2[:])
            for cj in range(4):
                ps = pp.tile([P, 512], dt.float32, tag="a", bufs=2,
                             name=f"psV_{tt}_{cj}")
                for kc in range(KC):
                    nc.tensor.matmul(
                        ps[:], xt[:, kc, cj * P:(cj + 1) * P],
                        wv_sb[:, kc, :],
                        start=(kc == 0), stop=(kc == KC - 1))
                nc.scalar.copy(v_t[tt][:, cj, :], ps[:])

            # gather the previous round's A2A results now: issued on the
            # in-order SP queue before att(tt)'s stage DMAs
            if tt >= 1:
                yh_pre = [load_half(tt - 1, 0), load_half(tt - 1, 1)]

            # prefetch next tile's x/cos/sin now: issued on the in-order SP
            # queue BEFORE att(tt)'s stage DMAs, so they are not gated on
            # this round's normalization
            if tt + 1 < QT:
                nsl = slice((tt + 1) * 512, (tt + 2) * 512)
                xtn = xp.tile([P, KC, 512], BF, tag="xt", name=f"xt_{tt+1}")
                nc.sync.dma_start(xtn[:], xT_view[:, :, nsl])
                ccn = csp.tile([P, 512], dt.float32, tag="cc",
                               name=f"cc_{tt+1}")
                nc.sync.dma_start(ccn[:], cosA_d.ap()[:, nsl])
                ssn = csp.tile([P, 512], dt.float32, tag="ss",
                               name=f"ss_{tt+1}")
                nc.sync.dma_start(ssn[:], sinA_d.ap()[:, nsl])
                nxt[tt + 1] = (xtn, ccn, ssn)

            # ---- att(tt): causal attention for queries of this tile ----
            n = 4 * (tt + 1)
            for h in range(HPC):
                qmv = qtile[:, h, :]
                smps = pp.tile([P, 512], dt.float32, tag="sm", bufs=1,
                               name=f"sm_{tt}_{h}")
                pvps = pp.tile([P, 512], dt.float32, tag="pv", bufs=2,
                               name=f"pv_{tt}_{h}")
                sc_tiles = {}

                def emit_sc(jc, _h=h, _q=qmv, _tt=tt, _sc=sc_tiles):
                    ps = pp.tile([P, 512], dt.float32, tag="sc", bufs=2,
                                 name=f"sc_{_tt}_{_h}_{jc}")
                    nc.tensor.matmul(
                        ps[:], k_t[jc // 4][:, _h, (jc % 4) * P:(jc % 4 + 1) * P],
                        _q, start=True, stop=True)
                    _sc[jc] = ps

                emit_sc(0)
                if n > 1:
                    emit_sc(1)
                for jc in range(n):
                    scps = sc_tiles.pop(jc)
                    pT = ptp.tile([P, 512], BF, tag="pT", bufs=3,
                                  name=f"pT_{tt}_{h}_{jc}")
                    nc.scalar.activation(pT[:], scps[:],
                                         mybir.ActivationFunctionType.Exp,
                                         scale=SCALE)
                    if jc >= n - 4:
                        o = jc - (n - 4)
                        pTm = ptp.tile([P, 512], BF, tag="pTm", bufs=1,
                                       name=f"pTm_{tt}_{h}_{jc}")
                        nc.vector.tensor_mul(
                            pTm[:], pT[:],
                            binm[:, (3 - o) * 128:(3 - o) * 128 + 512])
                        pTu = pTm
                    else:
                        pTu = pT
                    if jc + 2 < n:
                        emit_sc(jc + 2)
                    nc.tensor.matmul(smps[:], ones_r[:], pTu[:],
                                     start=(jc == 0), stop=(jc == n - 1))
                    nc.tensor.matmul(pvps[:],
                                     v_t[jc // 4][:, jc % 4, h * P:(h + 1) * P],
                                     pTu[:],
                                     start=(jc == 0), stop=(jc == n - 1))

                # deferred softmax normalization
                rec = tp.tile([P, 512], dt.float32, tag="tr", bufs=3,
                              name=f"rec_{tt}_{h}")
                nc.vector.reciprocal_approx_fast(rec[0:1, :], smps[0:1, :])
                bc = tp.tile([P, 512], dt.float32, tag="tr", bufs=3,
                             name=f"bc_{tt}_{h}")
                nc.gpsimd.partition_broadcast(bc[:], rec[0:1, :])
                yt = yp.tile([P, 512], BF, tag="yt", bufs=1,
                             name=f"yt_{tt}_{h}")
                nc.vector.tensor_mul(yt[:], pvps[:], bc[:])
                if tt < QT - 1:
                    nc.sync.dma_start(
                        chs[tt][:, h * P:(h + 1) * P, :]
                        .rearrange("s d t -> d s t"),
                        yt[:].rearrange("d (s t) -> d s t", s=NCORES))
                else:
                    nc.sync.dma_start(
                        ch3[h // 2][:, (h % 2) * P:(h % 2 + 1) * P, :]
                        .rearrange("s d t -> d s t"),
                        yt[:].rearrange("d (s t) -> d s t", s=NCORES))
                    if h % 2 == 1:
                        nc.gpsimd.collective_compute(
                            "AllToAll", mybir.AluOpType.bypass,
                            replica_groups=[list(range(NCORES))],
                            ins=[ch3[h // 2].opt()], outs=[yo3[h // 2].opt()],
                        )
                # mid-round: previous round's output projection fills the PE
                # while this round's (and the last half-)A2A is in flight,
                # and its PSUM drain happens while act is otherwise idle
                if h == 1 and tt >= 1:
                    emit_C(tt - 1, pre=yh_pre)
                if h == 1 and tt == QT - 1:
                    # issue the last round's first-half gathers ahead of
                    # h2/h3's stage DMAs on the in-order SP queue (after
                    # C(2)'s own gathers to keep the yh tag rotation acyclic)
                    yh30 = load_half(QT - 1, 0)

            if tt < QT - 1:
                nc.gpsimd.collective_compute(
                    "AllToAll", mybir.AluOpType.bypass,
                    replica_groups=[list(range(NCORES))],
                    ins=[chs[tt].opt()], outs=[yos[tt].opt()],
                )
        emit_C(QT - 1, pre=[yh30])


    nc.compile()
    return nc


def _host_reference(x, weights, cos, sin, mask, use_lora):
    """Numpy fallback for inputs outside the optimized assumptions."""
    (q_w, q_b, q_A, q_B, k_w, k_b, k_A, k_B,
     v_w, v_b, v_A, v_B, p_w, p_b, p_A, p_B) = weights

    def lin(xx, w, b, A, Bm):
        out = xx @ w.T + b
        if use_lora:
            out = out + (xx @ A) @ Bm
        return out

    def rope(t):
        x1, x2 = t[..., ::2], t[..., 1::2]
        y = np.stack((x1 * cos - x2 * sin, x1 * sin + x2 * cos), axis=-1)
        return y.reshape(t.shape)

    Bs, Tl, Cd = x.shape
    q = lin(x, q_w, q_b, q_A, q_B).reshape(Bs, Tl, H, D).transpose(0, 2, 1, 3)
    k = lin(x, k_w, k_b, k_A, k_B).reshape(Bs, Tl, H, D).transpose(0, 2, 1, 3)
    v = lin(x, v_w, v_b, v_A, v_B).reshape(Bs, Tl, H, D).transpose(0, 2, 1, 3)
    q, k = rope(q), rope(k)
    s = np.einsum('bhqd,bhkd->bhqk', q, k) / np.sqrt(D)
    s = np.where(mask, s, -np.inf)
    s = s - s.max(axis=-1, keepdims=True)
    p = np.exp(s)
    p /= p.sum(axis=-1, keepdims=True)
    o = np.einsum('bhqk,bhkd->bhqd', p, v).transpose(0, 2, 1, 3).reshape(Bs, Tl, Cd)
    return lin(o, p_w, p_b, p_A, p_B).astype(np.float32)


def kernel(**inputs):
    x = np.asarray(inputs["x"], np.float32)
    cos = np.asarray(inputs["cos"], np.float32)
    sin = np.asarray(inputs["sin"], np.float32)
    mask = np.asarray(inputs["mask"])
    use_lora = int(np.asarray(inputs["use_lora"]))
    ws = {}
    for nm in ("q", "k", "v", "p"):
        for suf in ("w", "b", "A", "B"):
            ws[f"{nm}_{suf}"] = np.asarray(inputs[f"{nm}_{suf}"], np.float32)

    causal = bool((mask == np.tril(np.ones((T, T), bool))).all())
    zero_bias = all(not ws[f"{nm}_b"].any() for nm in ("q", "k", "v", "p"))
    if not (causal and zero_bias and x.shape == (B, T, C)):
        weights = tuple(ws[f"{nm}_{suf}"] for nm in ("q", "k", "v", "p")
                        for suf in ("w", "b", "A", "B"))
        return _host_reference(x, weights, cos, sin, mask, use_lora)

    bf = ml_dtypes.bfloat16

    # effective (LoRA-folded) transposed weights: out = x @ W_eff.T,
    # W_eff.T = w.T + A @ B
    effT = {}
    for nm in ("q", "k", "v", "p"):
        wt = ws[f"{nm}_w"].T.copy()
        if use_lora:
            wt += ws[f"{nm}_A"] @ ws[f"{nm}_B"]
        effT[nm] = wt

    # sigma: within each head reorder q/k out-features to [evens, odds] so
    # the rope pair-rotation becomes a partition half-swap
    perm = np.concatenate([np.arange(0, D, 2), np.arange(1, D, 2)])
    cosT = cos.T.astype(np.float32)          # [64, T]
    sinT = sin.T.astype(np.float32)
    cosA = np.ascontiguousarray(np.vstack([cosT, cosT]))
    sinA = np.ascontiguousarray(np.vstack([-sinT, sinT]))

    # 0/1 causal mask window table: mask for diagonal offset o is
    # binm[:, (3-o)*128 : (3-o)*128+512]  (mask[j, q] = j + 128*o <= q)
    jj = np.arange(P)[:, None]
    ww = np.arange(896)[None, :]
    binm = (jj <= ww - 384).astype(bf)

    # output projection weight, blocked [p_ci, kc, co]
    pwM = np.ascontiguousarray(
        effT["p"].reshape(KC, P, C).transpose(1, 0, 2)).astype(bf)

    global _PROGRAM
    if _PROGRAM is None:
        _PROGRAM = _build_program()
    nc = _PROGRAM

    in_maps = []
    for c in range(NCORES):
        b, hg = c // 4, c % 4
        cols = slice(hg * HPC * D, (hg + 1) * HPC * D)
        wqT = effT["q"][:, cols].copy()
        wkT = effT["k"][:, cols].copy()
        for hl in range(HPC):
            sl = slice(hl * D, (hl + 1) * D)
            wqT[:, sl] = wqT[:, sl][:, perm]
            wkT[:, sl] = wkT[:, sl][:, perm]
        in_maps.append({
            "xT": np.ascontiguousarray(x[b].T).astype(bf),
            "wqT": np.ascontiguousarray(wqT).astype(bf),
            "wkT": np.ascontiguousarray(wkT).astype(bf),
            "wvT": np.ascontiguousarray(effT["v"][:, cols]).astype(bf),
            "pwM": pwM,
            "cosA": cosA,
            "sinA": sinA,
            "binm": binm,
        })

    res = run_bass_kernel_spmd(nc, in_maps, list(range(NCORES)))

    out = np.empty((B, T, C), np.float32)
    for c in range(NCORES):
        oN = res.results[c]["outN"]                    # [512, 2048]
        blk = oN.reshape(QT, 2, 64, C)
        for r in range(QT):
            t0 = r * 512 + c * 64
            out[0, t0:t0 + 64, :] = blk[r, 0]
            out[1, t0:t0 + 64, :] = blk[r, 1]
    return out
